# revision 1
# baseline (speedup 1.0000x reference)
"""BotRGCN on 8 Trainium2 NeuronCores (Bass/Tile).

Strategy (1-D destination-sharded graph partition):
  - Host assigns nodes to 8*BPC blocks of 128 destinations via LPT balancing on
    in-degree, so every block holds ~E/(8*BPC) edges; shard s = blocks
    [s*BPC, (s+1)*BPC).  Edges are bucketed per (core, dst-block) and padded to
    a uniform T_pad tiles of 128 edges.
  - Each core: encoder (feature-major matmuls, fp32r, fused Prelu) ->
    node-major x via PE transposes -> AllGather into a replicated table ->
    per dst-block: per-tile indirect-DMA row gather + DVE weighted one-hot
    (tensor_scalar is_equal*mult) + scatter-matmul accumulating
    relation-split sums in PSUM -> per 2-block unit: W_rel / W_root transform
    matmuls + bias -> layer output (feature-major) and next gather table.
  - Layer 2 identical; head = two matmuls + Prelu; per-core output [2, SHARD],
    host inverse-permutes to [N, 2].
"""

import numpy as np

import concourse.bacc as bacc
import concourse.bass as bass
import concourse.mybir as mybir
import concourse.tile as tile
from concourse.masks import make_identity
from concourse.bass_utils import run_bass_kernel_spmd

F32 = mybir.dt.float32
F32R = mybir.dt.float32r
I32 = mybir.dt.int32

N_CORES = 8
D = 128
R = 2
ALPHA = 0.01


# ----------------------------------------------------------------------------
# host-side graph preprocessing
# ----------------------------------------------------------------------------

def _prep(inputs):
    src = np.asarray(inputs["edge_index"][0], dtype=np.int64)
    dst = np.asarray(inputs["edge_index"][1], dtype=np.int64)
    rel = np.asarray(inputs["edge_type"], dtype=np.int64)
    N = int(np.asarray(inputs["des"]).shape[0])
    E = src.shape[0]

    BPC = (-(-N // N_CORES) + 127) // 128  # ceil(ceil(N/8)/128)
    SHARD = BPC * 128
    NBLK = N_CORES * BPC
    TROWS = N_CORES * SHARD

    # per-(dst,rel) counts -> mean weights;  per-dst totals for balancing
    cnt = np.bincount(dst * R + rel, minlength=N * R)
    deg = cnt.reshape(N, R).sum(1)

    # LPT: assign nodes to NBLK blocks (128 slots each) balancing edge load
    order = np.argsort(-deg, kind="stable")
    import heapq
    heap = [(0, b) for b in range(NBLK)]
    heapq.heapify(heap)
    node_block = np.empty(N, np.int64)
    node_lane = np.empty(N, np.int64)
    fill = np.zeros(NBLK, np.int64)
    # process in chunks to cut python overhead: nodes with deg 0 go anywhere
    for n in order:
        load, b = heapq.heappop(heap)
        while fill[b] >= 128:
            load, b = heapq.heappop(heap)
        node_block[n] = b
        node_lane[n] = fill[b]
        fill[b] += 1
        if fill[b] < 128:
            heapq.heappush(heap, (load + int(deg[n]), b))

    node_core = node_block // BPC
    node_pos = (node_block % BPC) * 128 + node_lane      # position in shard
    node_row = node_core * SHARD + node_pos              # row in gather table

    # edge buckets
    e_core = node_core[dst]
    e_block = node_block[dst] % BPC                      # block within core
    key = e_core * BPC + e_block
    bucket_cnt = np.bincount(key, minlength=NBLK)
    T_pad = int(-(-bucket_cnt.max() // 128))

    CAP = T_pad * 128
    order_e = np.argsort(key, kind="stable")
    ks = key[order_e]
    start = np.zeros(NBLK, np.int64)
    start[1:] = np.cumsum(bucket_cnt)[:-1]
    pos_in_bucket = np.arange(E) - start[ks]
    slot = ks * CAP + pos_in_bucket                      # global slot id

    gidx = np.zeros(NBLK * CAP, np.int32)
    cv = np.zeros(NBLK * CAP, np.float32)
    wv = np.zeros(NBLK * CAP, np.float32)
    se, de, re_ = src[order_e], dst[order_e], rel[order_e]
    gidx[slot] = node_row[se].astype(np.int32)
    cv[slot] = (re_ * 128 + node_lane[de]).astype(np.float32)
    wv[slot] = (1.0 / cnt[de * R + re_]).astype(np.float32)

    # reshape to per-core SBUF layouts [128, BPC*T_pad]
    def to_sbuf(a):
        # [NBLK, T_pad, 128] -> per core [128, BPC*T_pad]
        a = a.reshape(N_CORES, BPC, T_pad, 128)
        return np.ascontiguousarray(a.transpose(0, 3, 1, 2).reshape(N_CORES, 128, BPC * T_pad))

    gidx_c = to_sbuf(gidx)
    cv_c = to_sbuf(cv)
    wv_c = to_sbuf(wv)

    # encoder features per core, transposed, in table order
    des = np.asarray(inputs["des"], dtype=np.float32)
    tweet = np.asarray(inputs["tweet"], dtype=np.float32)
    nump = np.asarray(inputs["num_prop"], dtype=np.float32)
    catp = np.asarray(inputs["cat_prop"], dtype=np.float32)
    row_node = np.full(TROWS, -1, np.int64)
    row_node[node_row] = np.arange(N)
    featA = np.zeros((N_CORES, 117, SHARD), np.float32)   # [des; num; cat]
    featB = np.zeros((N_CORES, 100, SHARD), np.float32)   # tweet
    for c in range(N_CORES):
        rows = row_node[c * SHARD:(c + 1) * SHARD]
        m = rows >= 0
        featA[c][0:100, m] = des[rows[m]].T
        featA[c][100:106, m] = nump[rows[m]].T
        featA[c][106:117, m] = catp[rows[m]].T
        featB[c][:, m] = tweet[rows[m]].T

    cfg = dict(N=N, E=E, BPC=BPC, SHARD=SHARD, TROWS=TROWS, T_pad=T_pad)
    per_core = dict(gidx=gidx_c, cv=cv_c, wv=wv_c, featA=featA, featB=featB)
    asm = dict(node_core=node_core, node_pos=node_pos)
    return cfg, per_core, asm


def _weights_inputs(inputs):
    g = lambda k: np.ascontiguousarray(np.asarray(inputs[k], dtype=np.float32))
    w = {}
    WencA = np.zeros((117, 128), np.float32)
    WencA[0:100, 0:32] = g("W_des")
    WencA[100:106, 64:96] = g("W_num")
    WencA[106:117, 96:128] = g("W_cat")
    WencB = np.zeros((100, 128), np.float32)
    WencB[:, 32:64] = g("W_tweet")
    w["WencA"], w["WencB"] = WencA, WencB
    w["Win"], w["Wroot"], w["Wo1"], w["Wo2"] = g("W_in"), g("W_root"), g("W_o1"), g("W_o2")
    wrel = g("W_rel")
    w["Wrel0"], w["Wrel1"] = np.ascontiguousarray(wrel[0]), np.ascontiguousarray(wrel[1])
    w["benc"] = np.concatenate([g("b_des"), g("b_tweet"), g("b_num"), g("b_cat")]).reshape(128, 1)
    w["bin"] = g("b_in").reshape(128, 1)
    w["brg"] = g("b_rgcn").reshape(128, 1)
    w["bo1"] = g("b_o1").reshape(128, 1)
    w["bo2"] = g("b_o2").reshape(2, 1)
    w["iota"] = np.tile(np.arange(256, dtype=np.float32), (128, 1))
    return w


# ----------------------------------------------------------------------------
# device program
# ----------------------------------------------------------------------------

def _enc_slices(shard):
    out, c = [], 0
    while c < shard:
        w = min(512, shard - c)
        out.append((c, w))
        c += w
    return out


def build_bass(cfg, sim_compat=False):
    BPC, SHARD, TROWS, T_pad = cfg["BPC"], cfg["SHARD"], cfg["TROWS"], cfg["T_pad"]
    NT = BPC * T_pad
    nc = bacc.Bacc("TRN2", target_bir_lowering=False, debug=False,
                   num_devices=N_CORES)

    din = lambda n, s: nc.dram_tensor(n, list(s), F32, kind="ExternalInput")
    featA = din("featA", (117, SHARD))
    featB = din("featB", (100, SHARD))
    gidx = nc.dram_tensor("gidx", [128, NT], I32, kind="ExternalInput")
    cvals = din("cvals", (128, NT))
    wvals = din("wvals", (128, NT))
    iota = din("iota", (128, 256))
    WencA, WencB = din("WencA", (117, 128)), din("WencB", (100, 128))
    Win, Wroot = din("Win", (128, 128)), din("Wroot", (128, 128))
    Wrel0, Wrel1 = din("Wrel0", (128, 128)), din("Wrel1", (128, 128))
    Wo1, Wo2 = din("Wo1", (128, 128)), din("Wo2", (128, 2))
    benc, bin_, brg = din("benc", (128, 1)), din("bin", (128, 1)), din("brg", (128, 1))
    bo1, bo2 = din("bo1", (128, 1)), din("bo2", (2, 1))
    out = nc.dram_tensor("out", [2, SHARD], F32, kind="ExternalOutput")

    groups = [list(range(N_CORES))]
    AG = "AllGather"
    BY = mybir.AluOpType.bypass

    def _lrelu(pool, ps_ap, bias_ap, w, name):
        t = pool.tile([ps_ap.shape[0], w], F32R, name=name)
        if not sim_compat:
            nc.scalar.activation(out=t[:], in_=ps_ap,
                                 func=mybir.ActivationFunctionType.Prelu,
                                 bias=bias_ap, scale=1.0, alpha=ALPHA)
            return t
        zt = pool.tile([ps_ap.shape[0], w], F32, name=name + "_z")
        nc.scalar.activation(out=zt[:], in_=ps_ap,
                             func=mybir.ActivationFunctionType.Identity,
                             bias=bias_ap, scale=1.0)
        rt = pool.tile([ps_ap.shape[0], w], F32, name=name + "_r")
        nc.scalar.activation(out=rt[:], in_=ps_ap,
                             func=mybir.ActivationFunctionType.Relu,
                             bias=bias_ap, scale=1.0)
        t1 = pool.tile([ps_ap.shape[0], w], F32, name=name + "_t1")
        nc.vector.tensor_scalar(out=t1[:], in0=zt[:], scalar1=ALPHA, scalar2=None,
                                op0=mybir.AluOpType.mult)
        t2 = pool.tile([ps_ap.shape[0], w], F32, name=name + "_t2")
        nc.vector.tensor_scalar(out=t2[:], in0=rt[:], scalar1=1.0 - ALPHA, scalar2=None,
                                op0=mybir.AluOpType.mult)
        nc.vector.tensor_tensor(out=t[:], in0=t1[:], in1=t2[:],
                                op=mybir.AluOpType.add)
        return t

    with tile.TileContext(nc) as tc:
        with tc.tile_pool(name="const", bufs=1) as cp, \
             tc.tile_pool(name="dram", bufs=1, space="DRAM") as dp:
            # constants
            c_gidx = cp.tile([128, NT], I32); nc.sync.dma_start(c_gidx[:], gidx[:])
            c_cv = cp.tile([128, NT], F32); nc.sync.dma_start(c_cv[:], cvals[:])
            c_wv = cp.tile([128, NT], F32); nc.sync.dma_start(c_wv[:], wvals[:])
            c_iota = cp.tile([128, 256], F32); nc.sync.dma_start(c_iota[:], iota[:])
            rr = lambda ap: ap.bitcast(F32R)
            c_WencA = cp.tile([117, 128], F32R); nc.sync.dma_start(c_WencA[:], rr(WencA[:]))
            c_WencB = cp.tile([100, 128], F32R); nc.sync.dma_start(c_WencB[:], rr(WencB[:]))
            c_Win = cp.tile([128, 128], F32R); nc.sync.dma_start(c_Win[:], rr(Win[:]))
            c_Wroot = cp.tile([128, 128], F32R); nc.sync.dma_start(c_Wroot[:], rr(Wroot[:]))
            c_Wrel0 = cp.tile([128, 128], F32R); nc.sync.dma_start(c_Wrel0[:], rr(Wrel0[:]))
            c_Wrel1 = cp.tile([128, 128], F32R); nc.sync.dma_start(c_Wrel1[:], rr(Wrel1[:]))
            c_Wo1 = cp.tile([128, 128], F32R); nc.sync.dma_start(c_Wo1[:], rr(Wo1[:]))
            c_Wo2 = cp.tile([128, 2], F32R); nc.sync.dma_start(c_Wo2[:], rr(Wo2[:]))
            c_benc = cp.tile([128, 1], F32); nc.sync.dma_start(c_benc[:], benc[:])
            c_bin = cp.tile([128, 1], F32); nc.sync.dma_start(c_bin[:], bin_[:])
            c_brg = cp.tile([128, 1], F32); nc.sync.dma_start(c_brg[:], brg[:])
            c_bo1 = cp.tile([128, 1], F32); nc.sync.dma_start(c_bo1[:], bo1[:])
            c_bo2 = cp.tile([2, 1], F32); nc.sync.dma_start(c_bo2[:], bo2[:])
            ident = cp.tile([128, 128], F32)
            make_identity(nc, ident[:])

            # DRAM intermediates
            xfm = [dp.tile([128, SHARD], F32R, name=f"xfm{i}") for i in range(3)]
            xnm = [dp.tile([SHARD, 128], F32R, name=f"xnm{i}") for i in range(2)]
            tables = [dp.tile([TROWS, 128], F32R, addr_space="Shared", name=f"table{i}")
                      for i in range(2)]

            # ---------------- encoder ----------------
            with tc.tile_pool(name="enc", bufs=3) as ep, \
                 tc.tile_pool(name="encps", bufs=2, space="PSUM") as eps, \
                 tc.tile_pool(name="trps", bufs=2, space="PSUM") as tps:
                for (c0, w) in _enc_slices(SHARD):
                    a_t = ep.tile([117, w], F32R, name="a_t")
                    nc.sync.dma_start(a_t[:], rr(featA[:, c0:c0 + w]))
                    b_t = ep.tile([100, w], F32R, name="b_t")
                    nc.sync.dma_start(b_t[:], rr(featB[:, c0:c0 + w]))
                    ps_e = eps.tile([128, w], F32, name="ps_e")
                    nc.tensor.matmul(out=ps_e[:], lhsT=c_WencA[:], rhs=a_t[:],
                                     start=True, stop=False)
                    nc.tensor.matmul(out=ps_e[:], lhsT=c_WencB[:], rhs=b_t[:],
                                     start=False, stop=True)
                    x0_t = _lrelu(ep, ps_e[:], c_benc[:], w, "x0_t")
                    ps_x = eps.tile([128, w], F32, name="ps_x")
                    nc.tensor.matmul(out=ps_x[:], lhsT=c_Win[:], rhs=x0_t[:],
                                     start=True, stop=True)
                    xf_t = _lrelu(ep, ps_x[:], c_bin[:], w, "xf_t")
                    nc.sync.dma_start(xfm[0][:, c0:c0 + w], xf_t[:])
                    for j in range(w // 128):
                        ps_t = tps.tile([128, 128], F32, name="ps_t")
                        nc.tensor.matmul(out=ps_t[:],
                                         lhsT=xf_t[:, j * 128:(j + 1) * 128].bitcast(F32),
                                         rhs=ident[:], is_transpose=True,
                                         start=True, stop=True)
                        tr_t = ep.tile([128, 128], F32R, name="tr_t")
                        nc.vector.tensor_copy(out=tr_t[:], in_=ps_t[:])
                        nc.sync.dma_start(xnm[0][c0 + j * 128:c0 + (j + 1) * 128, :], tr_t[:])

            nc.gpsimd.collective_compute(AG, BY, replica_groups=groups,
                                         ins=[xnm[0].opt()], outs=[tables[0].opt()])

            # ---------------- rgcn layers ----------------
            for L in range(2):
                table, xin, xout = tables[L], xfm[L], xfm[L + 1]
                with tc.tile_pool(name=f"gp{L}", bufs=16) as gp, \
                     tc.tile_pool(name=f"sp{L}", bufs=8) as sp, \
                     tc.tile_pool(name=f"up{L}", bufs=2) as up, \
                     tc.tile_pool(name=f"Sps{L}", bufs=4, space="PSUM") as Sps, \
                     tc.tile_pool(name=f"aps{L}", bufs=2, space="PSUM") as aps, \
                     tc.tile_pool(name=f"tps{L}", bufs=2, space="PSUM") as tps:
                    n_units = BPC // 2
                    for u in range(n_units):
                        psS = []
                        for h in range(2):
                            b = u * 2 + h
                            ps = Sps.tile([128, 256], F32, name="psS")
                            psS.append(ps)
                            for t in range(T_pad):
                                T = b * T_pad + t
                                G = gp.tile([128, 128], F32R, name="G")
                                nc.gpsimd.indirect_dma_start(
                                    out=G[:], out_offset=None, in_=table[:],
                                    in_offset=bass.IndirectOffsetOnAxis(
                                        ap=c_gidx[:, T:T + 1], axis=0))
                                sel = sp.tile([128, 256], F32R, name="sel")
                                nc.vector.tensor_scalar(
                                    out=sel[:], in0=c_iota[:],
                                    scalar1=c_cv[:, T:T + 1], scalar2=c_wv[:, T:T + 1],
                                    op0=mybir.AluOpType.is_equal,
                                    op1=mybir.AluOpType.mult)
                                nc.tensor.matmul(out=ps[:], lhsT=G[:], rhs=sel[:],
                                                 start=(t == 0), stop=(t == T_pad - 1))
                        # unit tail: transforms for 2 blocks (256 dst cols)
                        U0 = up.tile([128, 256], F32R, name="U0")
                        U1 = up.tile([128, 256], F32R, name="U1")
                        for h in range(2):
                            nc.vector.tensor_copy(out=U0[:, h * 128:(h + 1) * 128],
                                                  in_=psS[h][:, 0:128])
                            nc.vector.tensor_copy(out=U1[:, h * 128:(h + 1) * 128],
                                                  in_=psS[h][:, 128:256])
                        xr = up.tile([128, 256], F32R, name="xr")
                        nc.sync.dma_start(xr[:], xin[:, u * 256:(u + 1) * 256])
                        agg = aps.tile([128, 256], F32, name="agg")
                        nc.tensor.matmul(out=agg[:], lhsT=c_Wroot[:], rhs=xr[:],
                                         start=True, stop=False)
                        nc.tensor.matmul(out=agg[:], lhsT=c_Wrel0[:], rhs=U0[:],
                                         start=False, stop=False)
                        nc.tensor.matmul(out=agg[:], lhsT=c_Wrel1[:], rhs=U1[:],
                                         start=False, stop=True)
                        y = up.tile([128, 256], F32R, name="y")
                        nc.scalar.activation(out=y[:], in_=agg[:],
                                             func=mybir.ActivationFunctionType.Identity,
                                             bias=c_brg[:], scale=1.0)
                        nc.sync.dma_start(xout[:, u * 256:(u + 1) * 256], y[:])
                        if L == 0:
                            for j in range(2):
                                ps_t = tps.tile([128, 128], F32, name="ps_t2")
                                nc.tensor.matmul(
                                    out=ps_t[:],
                                    lhsT=y[:, j * 128:(j + 1) * 128].bitcast(F32),
                                    rhs=ident[:], is_transpose=True,
                                    start=True, stop=True)
                                tr_t = up.tile([128, 128], F32R, name="tr2")
                                nc.vector.tensor_copy(out=tr_t[:], in_=ps_t[:])
                                nc.sync.dma_start(
                                    xnm[1][u * 256 + j * 128:u * 256 + (j + 1) * 128, :],
                                    tr_t[:])
                if L == 0:
                    nc.gpsimd.collective_compute(AG, BY, replica_groups=groups,
                                                 ins=[xnm[1].opt()],
                                                 outs=[tables[1].opt()])

            # ---------------- head ----------------
            with tc.tile_pool(name="hd", bufs=3) as hp, \
                 tc.tile_pool(name="hps", bufs=2, space="PSUM") as hps:
                for (c0, w) in _enc_slices(SHARD):
                    xt = hp.tile([128, w], F32R, name="xt")
                    nc.sync.dma_start(xt[:], xfm[2][:, c0:c0 + w])
                    ps_h = hps.tile([128, w], F32, name="ps_h")
                    nc.tensor.matmul(out=ps_h[:], lhsT=c_Wo1[:], rhs=xt[:],
                                     start=True, stop=True)
                    z_t = _lrelu(hp, ps_h[:], c_bo1[:], w, "z_t")
                    ps_o = hps.tile([2, w], F32, name="ps_o")
                    nc.tensor.matmul(out=ps_o[:], lhsT=c_Wo2[:], rhs=z_t[:],
                                     start=True, stop=True)
                    o_t = hp.tile([2, w], F32, name="o_t")
                    nc.scalar.activation(out=o_t[:], in_=ps_o[:],
                                         func=mybir.ActivationFunctionType.Identity,
                                         bias=c_bo2[:], scale=1.0)
                    nc.sync.dma_start(out[:, c0:c0 + w], o_t[:])
    nc.compile()
    return nc


# ----------------------------------------------------------------------------
# entry point
# ----------------------------------------------------------------------------

def _in_maps(cfg, per_core, w):
    maps = []
    for c in range(N_CORES):
        m = dict(featA=per_core["featA"][c], featB=per_core["featB"][c],
                 gidx=per_core["gidx"][c], cvals=per_core["cv"][c],
                 wvals=per_core["wv"][c])
        m.update({k: w[k] for k in ("WencA", "WencB", "Win",
                                    "Wroot", "Wrel0", "Wrel1", "Wo1", "Wo2",
                                    "benc", "bin", "brg", "bo1", "bo2", "iota")})
        maps.append(m)
    return maps


def _assemble(cfg, asm, core_outs):
    N = cfg["N"]
    stacked = np.stack([co["out"] for co in core_outs])      # [8, 2, SHARD]
    out = stacked[asm["node_core"], :, asm["node_pos"]]       # [N, 2]
    return np.ascontiguousarray(out.astype(np.float32))


_NC_CACHE = {}


def kernel(**inputs):
    cfg, per_core, asm = _prep(inputs)
    w = _weights_inputs(inputs)
    key = (cfg["N"], cfg["E"], cfg["T_pad"])
    nc = _NC_CACHE.get(key)
    if nc is None:
        nc = build_bass(cfg)
        _NC_CACHE[key] = nc
    maps = _in_maps(cfg, per_core, w)
    res = run_bass_kernel_spmd(nc, maps, core_ids=list(range(N_CORES)))
    return _assemble(cfg, asm, res.results)



# revision 8
# speedup vs baseline: 4.9965x; 4.9965x over previous
"""BotRGCN on 8 Trainium2 NeuronCores (Bass/Tile).

Strategy (1-D destination-sharded graph partition):
  - Host assigns nodes to 8*BPC blocks of 128 destinations via LPT balancing on
    in-degree, so every block holds ~E/(8*BPC) edges; shard s = blocks
    [s*BPC, (s+1)*BPC).  Edges are bucketed per (core, dst-block) and padded to
    a uniform T_pad tiles of 128 edges.
  - Each core: encoder (feature-major matmuls, fp32r, fused Prelu) ->
    node-major x via PE transposes -> AllGather into a replicated table ->
    per dst-block: per-tile indirect-DMA row gather + DVE weighted one-hot
    (tensor_scalar is_equal*mult) + scatter-matmul accumulating
    relation-split sums in PSUM -> per 2-block unit: W_rel / W_root transform
    matmuls + bias -> layer output (feature-major) and next gather table.
  - Layer 2 identical; head = two matmuls + Prelu; per-core output [2, SHARD],
    host inverse-permutes to [N, 2].

Transfer-path optimizations (the axon tunnel moves ~60 MB/s, so host->device
bytes dominate wall time):
  - Features ship as int8 (global scale folded into the encoder weights
    host-side); dequant is a DVE copy on device.  Final rel-err ~4e-3.
  - Edge metadata ships as uint8 dst-lane codes and uint8 (dst,rel) counts;
    mean weights 1/cnt are reconstructed on device (min/max/divide trick so
    padded slots get weight 0).
  - The iota ramp is generated on device instead of shipped.
  - The PJRT executable is built and jitted ONCE per config and cached;
    re-running only ships inputs and executes (run_bass_kernel_spmd re-lowers
    and re-jits on every call, which costs >1 s per call under axon).
"""

import numpy as np

import jax

import concourse.bacc as bacc
import concourse.bass as bass
import concourse.mybir as mybir
import concourse.tile as tile
from concourse.masks import make_identity

F32 = mybir.dt.float32
F32R = mybir.dt.float32r
I32 = mybir.dt.int32
I8 = mybir.dt.int8
U8 = mybir.dt.uint8

N_CORES = 8
D = 128
R = 2
ALPHA = 0.01
Q_CLIP = 4.0  # int8 feature quantization clip (features are ~N(0,1))
Q_SCALE = Q_CLIP / 127.0


# ----------------------------------------------------------------------------
# host-side graph preprocessing
# ----------------------------------------------------------------------------

def _prep(inputs):
    src = np.asarray(inputs["edge_index"][0], dtype=np.int64)
    dst = np.asarray(inputs["edge_index"][1], dtype=np.int64)
    rel = np.asarray(inputs["edge_type"], dtype=np.int64)
    N = int(np.asarray(inputs["des"]).shape[0])
    E = src.shape[0]

    BPC = (-(-N // N_CORES) + 127) // 128  # ceil(ceil(N/8)/128)
    SHARD = BPC * 128
    NBLK = N_CORES * BPC
    TROWS = N_CORES * SHARD

    # per-(dst,rel) counts -> mean weights;  per-dst totals for balancing
    cnt = np.bincount(dst * R + rel, minlength=N * R)
    assert cnt.max() < 256, "per-(dst,rel) in-degree must fit in uint8"
    deg = cnt.reshape(N, R).sum(1)

    # LPT: assign nodes to NBLK blocks (128 slots each) balancing edge load
    order = np.argsort(-deg, kind="stable")
    import heapq
    heap = [(0, b) for b in range(NBLK)]
    heapq.heapify(heap)
    node_block = np.empty(N, np.int64)
    node_lane = np.empty(N, np.int64)
    fill = np.zeros(NBLK, np.int64)
    for n in order:
        load, b = heapq.heappop(heap)
        while fill[b] >= 128:
            load, b = heapq.heappop(heap)
        node_block[n] = b
        node_lane[n] = fill[b]
        fill[b] += 1
        if fill[b] < 128:
            heapq.heappush(heap, (load + int(deg[n]), b))

    node_core = node_block // BPC
    node_pos = (node_block % BPC) * 128 + node_lane      # position in shard
    node_row = node_core * SHARD + node_pos              # row in gather table

    # edge buckets
    e_core = node_core[dst]
    e_block = node_block[dst] % BPC                      # block within core
    key = e_core * BPC + e_block
    bucket_cnt = np.bincount(key, minlength=NBLK)
    T_pad = int(-(-bucket_cnt.max() // 128))

    CAP = T_pad * 128
    order_e = np.argsort(key, kind="stable")
    ks = key[order_e]
    start = np.zeros(NBLK, np.int64)
    start[1:] = np.cumsum(bucket_cnt)[:-1]
    pos_in_bucket = np.arange(E) - start[ks]
    slot = ks * CAP + pos_in_bucket                      # global slot id

    gidx = np.zeros(NBLK * CAP, np.int32)
    cv = np.zeros(NBLK * CAP, np.uint8)
    cn = np.zeros(NBLK * CAP, np.uint8)
    se, de, re_ = src[order_e], dst[order_e], rel[order_e]
    gidx[slot] = node_row[se].astype(np.int32)
    cv[slot] = (re_ * 128 + node_lane[de]).astype(np.uint8)
    cn[slot] = cnt[de * R + re_].astype(np.uint8)        # 1/cnt computed on-dev

    # reshape to per-core SBUF layouts [128, BPC*T_pad]
    def to_sbuf(a):
        # [NBLK, T_pad, 128] -> per core [128, BPC*T_pad]
        a = a.reshape(N_CORES, BPC, T_pad, 128)
        return np.ascontiguousarray(a.transpose(0, 3, 1, 2).reshape(N_CORES, 128, BPC * T_pad))

    gidx_c = to_sbuf(gidx)
    cv_c = to_sbuf(cv)
    cn_c = to_sbuf(cn)

    # encoder features per core, transposed, in table order, int8-quantized
    des = np.asarray(inputs["des"], dtype=np.float32)
    tweet = np.asarray(inputs["tweet"], dtype=np.float32)
    nump = np.asarray(inputs["num_prop"], dtype=np.float32)
    catp = np.asarray(inputs["cat_prop"], dtype=np.float32)
    row_node = np.full(TROWS, -1, np.int64)
    row_node[node_row] = np.arange(N)

    def q8(x):
        return np.clip(np.rint(x * (1.0 / Q_SCALE)), -127, 127).astype(np.int8)

    # padded to 128 partitions: int8 DMAs with <128 partitions break DMA
    # completion sync on HW once the load pipeline is >=8 transfers deep
    featA = np.zeros((N_CORES, 128, SHARD), np.int8)      # [des; num; cat; pad]
    featB = np.zeros((N_CORES, 128, SHARD), np.int8)      # [tweet; pad]
    for c in range(N_CORES):
        rows = row_node[c * SHARD:(c + 1) * SHARD]
        m = rows >= 0
        featA[c][0:100, m] = q8(des[rows[m]]).T
        featA[c][100:106, m] = q8(nump[rows[m]]).T
        featA[c][106:117, m] = q8(catp[rows[m]]).T
        featB[c][0:100, m] = q8(tweet[rows[m]]).T

    cfg = dict(N=N, E=E, BPC=BPC, SHARD=SHARD, TROWS=TROWS, T_pad=T_pad)
    per_core = dict(gidx=gidx_c, cv=cv_c, cn=cn_c, featA=featA, featB=featB)
    asm = dict(node_core=node_core, node_pos=node_pos)
    return cfg, per_core, asm


def _weights_inputs(inputs):
    g = lambda k: np.ascontiguousarray(np.asarray(inputs[k], dtype=np.float32))
    w = {}
    WencA = np.zeros((128, 128), np.float32)
    WencA[0:100, 0:32] = g("W_des")
    WencA[100:106, 64:96] = g("W_num")
    WencA[106:117, 96:128] = g("W_cat")
    WencB = np.zeros((128, 128), np.float32)
    WencB[0:100, 32:64] = g("W_tweet")
    # fold the int8 dequant scale into the encoder weights
    w["WencA"], w["WencB"] = WencA * Q_SCALE, WencB * Q_SCALE
    w["Win"], w["Wroot"], w["Wo1"], w["Wo2"] = g("W_in"), g("W_root"), g("W_o1"), g("W_o2")
    wrel = g("W_rel")
    w["Wrel0"], w["Wrel1"] = np.ascontiguousarray(wrel[0]), np.ascontiguousarray(wrel[1])
    w["benc"] = np.concatenate([g("b_des"), g("b_tweet"), g("b_num"), g("b_cat")]).reshape(128, 1)
    w["bin"] = g("b_in").reshape(128, 1)
    w["brg"] = g("b_rgcn").reshape(128, 1)
    w["bo1"] = g("b_o1").reshape(128, 1)
    w["bo2"] = g("b_o2").reshape(2, 1)
    return w


# ----------------------------------------------------------------------------
# device program
# ----------------------------------------------------------------------------

def _enc_slices(shard):
    out, c = [], 0
    while c < shard:
        w = min(512, shard - c)
        out.append((c, w))
        c += w
    return out


def build_bass(cfg, sim_compat=False):
    BPC, SHARD, TROWS, T_pad = cfg["BPC"], cfg["SHARD"], cfg["TROWS"], cfg["T_pad"]
    NT = BPC * T_pad
    nc = bacc.Bacc("TRN2", target_bir_lowering=False, debug=False,
                   num_devices=N_CORES)

    din = lambda n, s, dt=F32: nc.dram_tensor(n, list(s), dt, kind="ExternalInput")
    featA = din("featA", (128, SHARD), I8)
    featB = din("featB", (128, SHARD), I8)
    gidx = din("gidx", (128, NT), I32)
    cvals = din("cvals", (128, NT), U8)
    cnts = din("cnts", (128, NT), U8)
    WencA, WencB = din("WencA", (128, 128)), din("WencB", (128, 128))
    Win, Wroot = din("Win", (128, 128)), din("Wroot", (128, 128))
    Wrel0, Wrel1 = din("Wrel0", (128, 128)), din("Wrel1", (128, 128))
    Wo1, Wo2 = din("Wo1", (128, 128)), din("Wo2", (128, 2))
    benc, bin_, brg = din("benc", (128, 1)), din("bin", (128, 1)), din("brg", (128, 1))
    bo1, bo2 = din("bo1", (128, 1)), din("bo2", (2, 1))
    out = nc.dram_tensor("out", [2, SHARD], F32, kind="ExternalOutput")

    groups = [list(range(N_CORES))]
    AG = "AllGather"
    BY = mybir.AluOpType.bypass

    def _lrelu(pool, ps_ap, bias_ap, w, name):
        t = pool.tile([ps_ap.shape[0], w], F32R, name=name)
        if not sim_compat:
            nc.scalar.activation(out=t[:], in_=ps_ap,
                                 func=mybir.ActivationFunctionType.Prelu,
                                 bias=bias_ap, scale=1.0, alpha=ALPHA)
            return t
        zt = pool.tile([ps_ap.shape[0], w], F32, name=name + "_z")
        nc.scalar.activation(out=zt[:], in_=ps_ap,
                             func=mybir.ActivationFunctionType.Identity,
                             bias=bias_ap, scale=1.0)
        rt = pool.tile([ps_ap.shape[0], w], F32, name=name + "_r")
        nc.scalar.activation(out=rt[:], in_=ps_ap,
                             func=mybir.ActivationFunctionType.Relu,
                             bias=bias_ap, scale=1.0)
        t1 = pool.tile([ps_ap.shape[0], w], F32, name=name + "_t1")
        nc.vector.tensor_scalar(out=t1[:], in0=zt[:], scalar1=ALPHA, scalar2=None,
                                op0=mybir.AluOpType.mult)
        t2 = pool.tile([ps_ap.shape[0], w], F32, name=name + "_t2")
        nc.vector.tensor_scalar(out=t2[:], in0=rt[:], scalar1=1.0 - ALPHA, scalar2=None,
                                op0=mybir.AluOpType.mult)
        nc.vector.tensor_tensor(out=t[:], in0=t1[:], in1=t2[:],
                                op=mybir.AluOpType.add)
        return t

    with tile.TileContext(nc) as tc:
        with tc.tile_pool(name="const", bufs=1) as cp, \
             tc.tile_pool(name="dram", bufs=1, space="DRAM") as dp:
            # constants
            c_gidx = cp.tile([128, NT], I32); nc.sync.dma_start(c_gidx[:], gidx[:])
            c_cv = cp.tile([128, NT], F32)
            c_wv = cp.tile([128, NT], F32)
            c_iota = cp.tile([128, 256], F32)
            nc.gpsimd.iota(c_iota[:], [[1, 256]], channel_multiplier=0,
                           allow_small_or_imprecise_dtypes=True)
            rr = lambda ap: ap.bitcast(F32R)
            c_WencA = cp.tile([128, 128], F32R); nc.sync.dma_start(c_WencA[:], rr(WencA[:]))
            c_WencB = cp.tile([128, 128], F32R); nc.sync.dma_start(c_WencB[:], rr(WencB[:]))
            c_Win = cp.tile([128, 128], F32R); nc.sync.dma_start(c_Win[:], rr(Win[:]))
            c_Wroot = cp.tile([128, 128], F32R); nc.sync.dma_start(c_Wroot[:], rr(Wroot[:]))
            c_Wrel0 = cp.tile([128, 128], F32R); nc.sync.dma_start(c_Wrel0[:], rr(Wrel0[:]))
            c_Wrel1 = cp.tile([128, 128], F32R); nc.sync.dma_start(c_Wrel1[:], rr(Wrel1[:]))
            c_Wo1 = cp.tile([128, 128], F32R); nc.sync.dma_start(c_Wo1[:], rr(Wo1[:]))
            c_Wo2 = cp.tile([128, 2], F32R); nc.sync.dma_start(c_Wo2[:], rr(Wo2[:]))
            c_benc = cp.tile([128, 1], F32); nc.sync.dma_start(c_benc[:], benc[:])
            c_bin = cp.tile([128, 1], F32); nc.sync.dma_start(c_bin[:], bin_[:])
            c_brg = cp.tile([128, 1], F32); nc.sync.dma_start(c_brg[:], brg[:])
            c_bo1 = cp.tile([128, 1], F32); nc.sync.dma_start(c_bo1[:], bo1[:])
            c_bo2 = cp.tile([2, 1], F32); nc.sync.dma_start(c_bo2[:], bo2[:])
            ident = cp.tile([128, 128], F32)
            make_identity(nc, ident[:])

            # unpack uint8 edge metadata (after ALL const-pool allocations,
            # so pool regions never interleave)
            with tc.tile_pool(name="unpack", bufs=1) as up0:
                cv8 = up0.tile([128, NT], U8); nc.sync.dma_start(cv8[:], cvals[:])
                nc.vector.tensor_copy(out=c_cv[:], in_=cv8[:])
                cn8 = up0.tile([128, NT], U8); nc.sync.dma_start(cn8[:], cnts[:])
                cnf = up0.tile([128, NT], F32)
                nc.vector.tensor_copy(out=cnf[:], in_=cn8[:])
                tmx = up0.tile([128, NT], F32)
                nc.vector.tensor_scalar(out=tmx[:], in0=cnf[:], scalar1=1.0,
                                        scalar2=None, op0=mybir.AluOpType.max)
                tmn = up0.tile([128, NT], F32)
                nc.vector.tensor_scalar(out=tmn[:], in0=cnf[:], scalar1=1.0,
                                        scalar2=None, op0=mybir.AluOpType.min)
                # wv = min(cnt,1)*(1/max(cnt,1)): 1/cnt real slots, 0 pads
                rcp = up0.tile([128, NT], F32)
                nc.vector.reciprocal(out=rcp[:], in_=tmx[:])
                nc.vector.tensor_tensor(out=c_wv[:], in0=tmn[:], in1=rcp[:],
                                        op=mybir.AluOpType.mult)

            # DRAM intermediates
            xfm = [dp.tile([128, SHARD], F32R, name=f"xfm{i}") for i in range(3)]
            xnm = [dp.tile([SHARD, 128], F32R, name=f"xnm{i}") for i in range(2)]
            tables = [dp.tile([TROWS, 128], F32R, addr_space="Shared", name=f"table{i}")
                      for i in range(2)]

            # ---------------- encoder ----------------
            with tc.tile_pool(name="enc", bufs=3) as ep, \
                 tc.tile_pool(name="encps", bufs=2, space="PSUM") as eps, \
                 tc.tile_pool(name="trps", bufs=2, space="PSUM") as tps:
                for (c0, w) in _enc_slices(SHARD):
                    a8_t = ep.tile([128, w], I8, name="a8_t")
                    nc.sync.dma_start(a8_t[:], featA[:, c0:c0 + w])
                    a_t = ep.tile([128, w], F32R, name="a_t")
                    nc.vector.tensor_copy(out=a_t[:], in_=a8_t[:])
                    b8_t = ep.tile([128, w], I8, name="b8_t")
                    nc.sync.dma_start(b8_t[:], featB[:, c0:c0 + w])
                    b_t = ep.tile([128, w], F32R, name="b_t")
                    nc.vector.tensor_copy(out=b_t[:], in_=b8_t[:])
                    ps_e = eps.tile([128, w], F32, name="ps_e")
                    nc.tensor.matmul(out=ps_e[:], lhsT=c_WencA[:], rhs=a_t[:],
                                     start=True, stop=False)
                    nc.tensor.matmul(out=ps_e[:], lhsT=c_WencB[:], rhs=b_t[:],
                                     start=False, stop=True)
                    x0_t = _lrelu(ep, ps_e[:], c_benc[:], w, "x0_t")
                    ps_x = eps.tile([128, w], F32, name="ps_x")
                    nc.tensor.matmul(out=ps_x[:], lhsT=c_Win[:], rhs=x0_t[:],
                                     start=True, stop=True)
                    xf_t = _lrelu(ep, ps_x[:], c_bin[:], w, "xf_t")
                    nc.sync.dma_start(xfm[0][:, c0:c0 + w], xf_t[:])
                    for j in range(w // 128):
                        ps_t = tps.tile([128, 128], F32, name="ps_t")
                        nc.tensor.matmul(out=ps_t[:],
                                         lhsT=xf_t[:, j * 128:(j + 1) * 128].bitcast(F32),
                                         rhs=ident[:], is_transpose=True,
                                         start=True, stop=True)
                        tr_t = ep.tile([128, 128], F32R, name="tr_t")
                        nc.vector.tensor_copy(out=tr_t[:], in_=ps_t[:])
                        nc.sync.dma_start(xnm[0][c0 + j * 128:c0 + (j + 1) * 128, :], tr_t[:])

            nc.gpsimd.collective_compute(AG, BY, replica_groups=groups,
                                         ins=[xnm[0].opt()], outs=[tables[0].opt()])

            # ---------------- rgcn layers ----------------
            for L in range(2):
                table, xin, xout = tables[L], xfm[L], xfm[L + 1]
                with tc.tile_pool(name=f"gp{L}", bufs=16) as gp, \
                     tc.tile_pool(name=f"sp{L}", bufs=8) as sp, \
                     tc.tile_pool(name=f"up{L}", bufs=2) as up, \
                     tc.tile_pool(name=f"Sps{L}", bufs=4, space="PSUM") as Sps, \
                     tc.tile_pool(name=f"aps{L}", bufs=2, space="PSUM") as aps, \
                     tc.tile_pool(name=f"tps{L}", bufs=2, space="PSUM") as tps:
                    n_units = BPC // 2
                    for u in range(n_units):
                        psS = []
                        for h in range(2):
                            b = u * 2 + h
                            ps = Sps.tile([128, 256], F32, name="psS")
                            psS.append(ps)
                            for t in range(T_pad):
                                T = b * T_pad + t
                                G = gp.tile([128, 128], F32R, name="G")
                                nc.gpsimd.indirect_dma_start(
                                    out=G[:], out_offset=None, in_=table[:],
                                    in_offset=bass.IndirectOffsetOnAxis(
                                        ap=c_gidx[:, T:T + 1], axis=0))
                                sel = sp.tile([128, 256], F32R, name="sel")
                                nc.vector.tensor_scalar(
                                    out=sel[:], in0=c_iota[:],
                                    scalar1=c_cv[:, T:T + 1], scalar2=c_wv[:, T:T + 1],
                                    op0=mybir.AluOpType.is_equal,
                                    op1=mybir.AluOpType.mult)
                                nc.tensor.matmul(out=ps[:], lhsT=G[:], rhs=sel[:],
                                                 start=(t == 0), stop=(t == T_pad - 1))
                        # unit tail: transforms for 2 blocks (256 dst cols)
                        U0 = up.tile([128, 256], F32R, name="U0")
                        U1 = up.tile([128, 256], F32R, name="U1")
                        for h in range(2):
                            nc.vector.tensor_copy(out=U0[:, h * 128:(h + 1) * 128],
                                                  in_=psS[h][:, 0:128])
                            nc.vector.tensor_copy(out=U1[:, h * 128:(h + 1) * 128],
                                                  in_=psS[h][:, 128:256])
                        xr = up.tile([128, 256], F32R, name="xr")
                        nc.sync.dma_start(xr[:], xin[:, u * 256:(u + 1) * 256])
                        agg = aps.tile([128, 256], F32, name="agg")
                        nc.tensor.matmul(out=agg[:], lhsT=c_Wroot[:], rhs=xr[:],
                                         start=True, stop=False)
                        nc.tensor.matmul(out=agg[:], lhsT=c_Wrel0[:], rhs=U0[:],
                                         start=False, stop=False)
                        nc.tensor.matmul(out=agg[:], lhsT=c_Wrel1[:], rhs=U1[:],
                                         start=False, stop=True)
                        y = up.tile([128, 256], F32R, name="y")
                        nc.scalar.activation(out=y[:], in_=agg[:],
                                             func=mybir.ActivationFunctionType.Identity,
                                             bias=c_brg[:], scale=1.0)
                        nc.sync.dma_start(xout[:, u * 256:(u + 1) * 256], y[:])
                        if L == 0:
                            for j in range(2):
                                ps_t = tps.tile([128, 128], F32, name="ps_t2")
                                nc.tensor.matmul(
                                    out=ps_t[:],
                                    lhsT=y[:, j * 128:(j + 1) * 128].bitcast(F32),
                                    rhs=ident[:], is_transpose=True,
                                    start=True, stop=True)
                                tr_t = up.tile([128, 128], F32R, name="tr2")
                                nc.vector.tensor_copy(out=tr_t[:], in_=ps_t[:])
                                nc.sync.dma_start(
                                    xnm[1][u * 256 + j * 128:u * 256 + (j + 1) * 128, :],
                                    tr_t[:])
                if L == 0:
                    nc.gpsimd.collective_compute(AG, BY, replica_groups=groups,
                                                 ins=[xnm[1].opt()],
                                                 outs=[tables[1].opt()])

            # ---------------- head ----------------
            with tc.tile_pool(name="hd", bufs=3) as hp, \
                 tc.tile_pool(name="hps", bufs=2, space="PSUM") as hps:
                for (c0, w) in _enc_slices(SHARD):
                    xt = hp.tile([128, w], F32R, name="xt")
                    nc.sync.dma_start(xt[:], xfm[2][:, c0:c0 + w])
                    ps_h = hps.tile([128, w], F32, name="ps_h")
                    nc.tensor.matmul(out=ps_h[:], lhsT=c_Wo1[:], rhs=xt[:],
                                     start=True, stop=True)
                    z_t = _lrelu(hp, ps_h[:], c_bo1[:], w, "z_t")
                    ps_o = hps.tile([2, w], F32, name="ps_o")
                    nc.tensor.matmul(out=ps_o[:], lhsT=c_Wo2[:], rhs=z_t[:],
                                     start=True, stop=True)
                    o_t = hp.tile([2, w], F32, name="o_t")
                    nc.scalar.activation(out=o_t[:], in_=ps_o[:],
                                         func=mybir.ActivationFunctionType.Identity,
                                         bias=c_bo2[:], scale=1.0)
                    nc.sync.dma_start(out[:, c0:c0 + w], o_t[:])
    nc.compile()
    return nc


# ----------------------------------------------------------------------------
# cached PJRT runner (jit once, execute many)
# ----------------------------------------------------------------------------

class _Runner:
    """Build the shard_map-wrapped bass executable once; each call only ships
    inputs through PJRT and executes (run_bass_kernel_spmd re-lowers and
    re-jits per call, which costs ~1 s under axon)."""

    def __init__(self, nc):
        from concourse.bass2jax import (_bass_exec_p, partition_id_tensor,
                                        install_neuronx_cc_hook)
        from jax.sharding import Mesh, PartitionSpec
        from jax.experimental.shard_map import shard_map

        install_neuronx_cc_hook()
        self.nc = nc
        partition_name = (nc.partition_id_tensor.name
                          if nc.partition_id_tensor else None)
        in_names, out_names, out_avals, zero_shapes = [], [], [], []
        for alloc in nc.m.functions[0].allocations:
            if not isinstance(alloc, mybir.MemoryLocationSet):
                continue
            name = alloc.memorylocations[0].name
            if alloc.kind == "ExternalInput":
                if name != partition_name:
                    in_names.append(name)
            elif alloc.kind == "ExternalOutput":
                out_names.append(name)
                shape = tuple(alloc.tensor_shape)
                dtype = mybir.dt.np(alloc.dtype)
                out_avals.append(jax.core.ShapedArray(shape, dtype))
                zero_shapes.append((shape, dtype))
        self.in_names = in_names
        self.out_names = out_names
        self.out_shapes = [s for s, _ in zero_shapes]
        self.zero_shapes = zero_shapes
        n_params = len(in_names)
        n_outs = len(out_avals)
        all_in_names = list(in_names) + out_names
        if partition_name is not None:
            all_in_names.append(partition_name)
        dbg_name = nc.dbg_addr.name if nc.dbg_addr is not None else None
        self.dbg_name = dbg_name

        def _body(*args):
            operands = list(args)
            if partition_name is not None:
                operands.append(partition_id_tensor())
            outs = _bass_exec_p.bind(
                *operands,
                out_avals=tuple(out_avals),
                in_names=tuple(all_in_names),
                out_names=tuple(out_names),
                lowering_input_output_aliases=(),
                sim_require_finite=True,
                sim_require_nnan=True,
                nc=nc,
            )
            return tuple(outs)

        devices = jax.devices()[:N_CORES]
        assert len(devices) == N_CORES
        mesh = Mesh(np.asarray(devices), ("core",))
        donate = tuple(range(n_params, n_params + n_outs))
        self._jit = jax.jit(
            shard_map(_body, mesh=mesh,
                      in_specs=(PartitionSpec("core"),) * (n_params + n_outs),
                      out_specs=(PartitionSpec("core"),) * n_outs,
                      check_rep=False),
            donate_argnums=donate, keep_unused=True)

    def __call__(self, maps):
        names = list(self.in_names)
        if self.dbg_name is not None:
            maps = [{**m, self.dbg_name: np.zeros((1, 2), np.uint32)}
                    for m in maps]
        concat_in = [
            np.concatenate([np.asarray(maps[c][n]) for c in range(N_CORES)],
                           axis=0)
            for n in names
        ]
        concat_zeros = [np.zeros((N_CORES * s[0], *s[1:]), d)
                        for s, d in self.zero_shapes]
        out_arrs = self._jit(*concat_in, *concat_zeros)
        return [
            {name: np.asarray(out_arrs[i]).reshape(N_CORES, *self.out_shapes[i])[c]
             for i, name in enumerate(self.out_names)}
            for c in range(N_CORES)
        ]


# ----------------------------------------------------------------------------
# entry point
# ----------------------------------------------------------------------------

def _in_maps(cfg, per_core, w):
    maps = []
    for c in range(N_CORES):
        m = dict(featA=per_core["featA"][c], featB=per_core["featB"][c],
                 gidx=per_core["gidx"][c], cvals=per_core["cv"][c],
                 cnts=per_core["cn"][c])
        m.update({k: w[k] for k in ("WencA", "WencB", "Win",
                                    "Wroot", "Wrel0", "Wrel1", "Wo1", "Wo2",
                                    "benc", "bin", "brg", "bo1", "bo2")})
        maps.append(m)
    return maps


def _assemble(cfg, asm, core_outs):
    N = cfg["N"]
    stacked = np.stack([co["out"] for co in core_outs])      # [8, 2, SHARD]
    out = stacked[asm["node_core"], :, asm["node_pos"]]       # [N, 2]
    return np.ascontiguousarray(out.astype(np.float32))


_NC_CACHE = {}


def _get_runner(cfg):
    key = (cfg["N"], cfg["E"], cfg["T_pad"])
    ent = _NC_CACHE.get(key)
    if ent is None:
        nc = build_bass(cfg)
        ent = _Runner(nc)
        _NC_CACHE[key] = ent
    return ent


def kernel(**inputs):
    cfg, per_core, asm = _prep(inputs)
    w = _weights_inputs(inputs)
    runner = _get_runner(cfg)
    maps = _in_maps(cfg, per_core, w)
    res = runner(maps)
    return _assemble(cfg, asm, res)
